# revision 6
# baseline (speedup 1.0000x reference)
"""Trainium2 Bass kernel for nn_Attention (dense transformer attention w/ QK-LayerNorm).

Sharding: sequence-parallel over 8 cores. Core c handles batch b = c//2,
token half h = c%2 (512 tokens). K/V are computed redundantly for the full
batch element on both cores of a pair (cheaper than the measured ~130us
pairwise AllGather); Q only for local tokens. No collectives.

Dataflow is transpose-free: Q/K are produced channel-major ([C, tok]) so the
QK-LayerNorm reductions over C become ones-vector matmuls on the partition
axis, and S^T = K_h Q_h^T tiles feed softmax-exp directly; V is produced
token-major with an appended ones-column per head so the PV matmul emits the
softmax denominator as an extra output row. All matmuls run in float32r
(TF32-rate on the PE at N>=256, fp32 storage).

Engine partition-access rule honored throughout: writes and ACT reads start
at 32-aligned partitions; only DVE reads use unaligned bases.
"""

import numpy as np

B, N, C = 4, 1024, 1024
H, D = 16, 64
LN_EPS = 1e-5
N_CORES = 8
TL = 512          # local tokens per core
KT = 8            # channel tiles of 128
SCALE = D ** -0.5

_COMPILED = None


def _build():
    import concourse.bacc as bacc
    import concourse.tile as tile
    import concourse.mybir as mybir

    F32 = mybir.dt.float32
    F32R = mybir.dt.float32r
    AF = mybir.ActivationFunctionType
    OP = mybir.AluOpType

    nc = bacc.Bacc("TRN2", target_bir_lowering=False, debug=False,
                   num_devices=N_CORES)

    # ---- I/O ----
    xT_d = nc.dram_tensor("xT", [C, N], F32R, kind="ExternalInput").ap()
    qkvwT_d = nc.dram_tensor("qkvwT", [C, 3 * C], F32R, kind="ExternalInput").ap()
    projwT_d = nc.dram_tensor("projwT", [C, C], F32R, kind="ExternalInput").ap()
    wsums_d = nc.dram_tensor("wsums", [128, 16], F32R, kind="ExternalInput").ap()
    params_d = nc.dram_tensor("params", [128, 56], F32, kind="ExternalInput").ap()
    bsum_d = nc.dram_tensor("bsumC", [2, 1], F32, kind="ExternalInput").ap()
    bv_d = nc.dram_tensor("bv", [1, C], F32R, kind="ExternalInput").ap()
    selc_d = nc.dram_tensor("selc", [2, 256], F32R, kind="ExternalInput").ap()
    out_d = nc.dram_tensor("out", [C, TL], F32, kind="ExternalOutput").ap()

    with tile.TileContext(nc) as tc:
        with tc.tile_pool(name="persist", bufs=1) as pers, \
             tc.tile_pool(name="sq", bufs=2) as sqp, \
             tc.tile_pool(name="small", bufs=1) as smallp, \
             tc.tile_pool(name="ps", bufs=8, space="PSUM") as ps:

            # ---------- persistent tiles ----------
            khat = pers.tile([128, KT * N], F32R, tag="khat")       # 4MB
            vful = pers.tile([128, 8 * 1040], F32R, tag="vful")     # 4.06MB
            qhat = pers.tile([128, KT * TL], F32R, tag="qhat")      # 2MB
            osb = pers.tile([128, KT * TL], F32R, tag="osb")        # 2MB
            rstdb = pers.tile([128, 2 * N + 2 * TL], F32R, tag="rstdb")  # 1.5MB

            params = smallp.tile([128, 56], F32, tag="params")
            wsums = smallp.tile([128, 16], F32R, tag="wsums")
            bsum = smallp.tile([2, 1], F32, tag="bsum")
            bv = smallp.tile([1, C], F32R, tag="bv")
            ones_c = smallp.tile([128, 1], F32R, tag="ones_c")
            ones_r = smallp.tile([1, 128], F32R, tag="ones_r")
            sel = smallp.tile([128, 4], F32R, tag="sel")  # e0=[1,0], e1=[0,1]
            # stats: column-packed [2, 512] slices, rows = (q, k)
            # slots: 0 sumsA 1 sumsB 2 ssqA 3 ssqB 4 mean 5 msq 6 rstd 7 mr
            st = smallp.tile([2, 8 * 512], F32R, tag="st")
            selc = smallp.tile([2, 256], F32R, tag="selc")
            ones_big = smallp.tile([128, 128], F32R, tag="ones_big")
            den4 = smallp.tile([128, 512], F32R, tag="den4")  # rows 0,32,64,96

            def sl(i):
                return st[:, i * 512:(i + 1) * 512]

            def prm(grp, kt):  # qn_w qn_b kn_w kn_b qb kb projb
                return params[:, grp * 8 + kt: grp * 8 + kt + 1]

            # ---------- load ----------
            nc.sync.dma_start(params[:], params_d[:])
            nc.sync.dma_start(wsums[:], wsums_d[:])
            nc.sync.dma_start(bsum[:], bsum_d[:])
            nc.sync.dma_start(bv[:], bv_d[:])
            nc.sync.dma_start(selc[:], selc_d[:])
            nc.vector.memset(ones_big[:].bitcast(F32), 1.0)
            nc.vector.memset(ones_c[:].bitcast(F32), 1.0)
            nc.vector.memset(ones_r[:].bitcast(F32), 1.0)
            nc.vector.memset(sel[:].bitcast(F32), 0.0)
            nc.vector.memset(sel[:, 0:1].bitcast(F32), 1.0)   # e0 col0
            nc.vector.memset(sel[:, 3:4].bitcast(F32), 1.0)   # e1 col1

            with tc.tile_pool(name="xp", bufs=1) as xpool, \
                 tc.tile_pool(name="wpool", bufs=8) as wpool:
                xT = xpool.tile([128, KT * N], F32R, tag="xT")      # 4MB
                xsrc = xT_d.rearrange("(a p) t -> p a t", p=128)
                nc.sync.dma_start(xT[:].rearrange("p (a t) -> p a t", a=KT), xsrc)

                # ---------- token sums of (q,k) ----------
                sums_ps = [ps.tile([2, 512], F32, tag="ps", name=f"sums_ps{i}")
                           for i in range(2)]
                for kt in range(KT):
                    for nh in range(2):
                        nc.tensor.matmul(
                            sums_ps[nh][:], wsums[:, kt * 2: kt * 2 + 2],
                            xT[:, kt * N + nh * 512: kt * N + (nh + 1) * 512],
                            start=(kt == 0), stop=(kt == KT - 1))
                nc.vector.tensor_copy(sl(0), sums_ps[0][:])
                nc.vector.tensor_copy(sl(1), sums_ps[1][:])

                # ssq accumulators (rows q,k): A=nh0(+q), B=nh1
                ssq_ps = [ps.tile([2, 512], F32, tag="ps", name=f"ssq_ps{i}")
                          for i in range(2)]

                # ---------- K phase ----------
                for grp in range(2):
                    wk = [wpool.tile([128, 512], F32R, tag="w",
                                     name=f"wk{grp}_{i}") for i in range(KT)]
                    for kt in range(KT):
                        nc.sync.dma_start(
                            wk[kt][:],
                            qkvwT_d[kt * 128:(kt + 1) * 128,
                                    C + grp * 512: C + (grp + 1) * 512])
                    for mi in range(4):
                        m = grp * 4 + mi
                        for nh in range(2):
                            acc = ps.tile([128, 512], F32, tag="ps")
                            for kt in range(KT):
                                nc.tensor.matmul(
                                    acc[:], wk[kt][:, mi * 128:(mi + 1) * 128],
                                    xT[:, kt * N + nh * 512: kt * N + (nh + 1) * 512],
                                    start=(kt == 0), stop=(kt == KT - 1))
                            dst = khat[:, m * N + nh * 512: m * N + (nh + 1) * 512]
                            nc.vector.tensor_scalar_add(dst, acc[:], prm(5, m))
                            ksq = sqp.tile([128, 512], F32R, tag="sq")
                            nc.vector.tensor_mul(ksq[:], dst, dst)
                            nc.tensor.matmul(ssq_ps[nh][:], sel[:, 2:4], ksq[:],
                                             start=(m == 0), stop=False,
                                             skip_group_check=True)

                # ---------- V phase ----------
                for nh in range(2):
                    wv = [wpool.tile([128, 512], F32R, tag="w",
                                     name=f"wv{nh}_{i}") for i in range(KT)]
                    for kt in range(KT):
                        nc.sync.dma_start(
                            wv[kt][:],
                            qkvwT_d[kt * 128:(kt + 1) * 128,
                                    2 * C + nh * 512: 2 * C + (nh + 1) * 512])
                    for mt in range(8):
                        base = mt * 1040
                        acc = ps.tile([128, 512], F32, tag="ps")
                        for kt in range(KT):
                            nc.tensor.matmul(
                                acc[:],
                                xT[:, kt * N + mt * 128: kt * N + (mt + 1) * 128],
                                wv[kt][:], start=(kt == 0), stop=False)
                        nc.tensor.matmul(acc[:], ones_r[:],
                                         bv[:, nh * 512:(nh + 1) * 512],
                                         start=False, stop=True)
                        dst = vful[:, base + nh * 8 * 65: base + (nh + 1) * 8 * 65]
                        nc.scalar.activation(
                            dst.rearrange("p (h e) -> p h e", h=8)[:, :, 0:64],
                            acc[:].rearrange("p (h e) -> p h e", h=8),
                            AF.Copy)
                for mt in range(8):
                    oc = vful[:, mt * 1040: (mt + 1) * 1040]
                    oc = oc.rearrange("p (h e) -> p h e", h=16)[:, :, 64:65]
                    nc.vector.memset(oc.bitcast(F32), 1.0)

                # ---------- Q phase ----------
                qsq = None
                for grp in range(2):
                    wq = [wpool.tile([128, 512], F32R, tag="w",
                                     name=f"wq{grp}_{i}") for i in range(KT)]
                    for kt in range(KT):
                        nc.sync.dma_start(
                            wq[kt][:],
                            qkvwT_d[kt * 128:(kt + 1) * 128,
                                    grp * 512:(grp + 1) * 512])
                    for mi in range(4):
                        m = grp * 4 + mi
                        acc = ps.tile([128, 512], F32, tag="ps")
                        for kt in range(KT):
                            nc.tensor.matmul(acc[:],
                                             wq[kt][:, mi * 128:(mi + 1) * 128],
                                             xT[:, kt * N: kt * N + TL],
                                             start=(kt == 0), stop=(kt == KT - 1))
                        dst = qhat[:, m * TL:(m + 1) * TL]
                        nc.vector.tensor_scalar_add(dst, acc[:], prm(4, m))
                        qsq = sqp.tile([128, 512], F32R, tag="sq")
                        nc.vector.tensor_mul(qsq[:], dst, dst)
                        nc.tensor.matmul(ssq_ps[0][:], sel[:, 0:2], qsq[:],
                                         start=False, stop=(m == 7),
                                         skip_group_check=True)
                # close ssq_ps[1] group (row 0 adds zeros)
                nc.tensor.matmul(ssq_ps[1][:], sel[:, 0:2], qsq[:],
                                 start=False, stop=True, skip_group_check=True)
                nc.vector.tensor_copy(sl(2), ssq_ps[0][:])
                nc.vector.tensor_copy(sl(3), ssq_ps[1][:])

                # ---------- stats chains (A: q + k-nh0, B: k-nh1) ----------
                def chain(sums_slot, ssq_slot):
                    nc.vector.tensor_scalar(sl(4), sums_slot, 1.0 / C,
                                            bsum[:], OP.mult, OP.add)
                    nc.vector.tensor_mul(sl(5), sl(4), sl(4))
                    nc.vector.tensor_scalar(sl(6), ssq_slot, 1.0 / C,
                                            LN_EPS, OP.mult, OP.add)
                    nc.vector.tensor_sub(sl(6), sl(6), sl(5))
                    nc.scalar.activation(sl(6), sl(6), AF.Ln)
                    nc.scalar.activation(sl(6), sl(6), AF.Exp, scale=-0.5)
                    nc.vector.tensor_mul(sl(7), sl(4), sl(6))

                def bcast(slot, row, dst_col):
                    bc_ps = ps.tile([128, 512], F32, tag="ps")
                    nc.tensor.matmul(bc_ps[:],
                                     selc[:, row * 128:(row + 1) * 128],
                                     sl(slot), start=True, stop=True)
                    nc.vector.tensor_copy(rstdb[:, dst_col:dst_col + 512],
                                          bc_ps[:])

                chain(sl(0), sl(2))                      # A: rows (q, k-nh0)
                bcast(6, 1, 0)                           # k rstd nh0
                bcast(7, 1, N)                           # k mr nh0
                bcast(6, 0, 2 * N)                       # q rstd
                bcast(7, 0, 2 * N + TL)                  # q mr
                chain(sl(1), sl(3))                      # B: rows (junk, k-nh1)
                bcast(6, 1, 512)                         # k rstd nh1
                bcast(7, 1, N + 512)                     # k mr nh1

                # ---------- normalize (in place) ----------
                for m in range(8):
                    s = khat[:, m * N:(m + 1) * N]
                    nc.vector.tensor_mul(s, s, rstdb[:, 0:N])
                    nc.vector.tensor_sub(s, s, rstdb[:, N:2 * N])
                    nc.vector.tensor_scalar(s, s, prm(2, m), prm(3, m),
                                            OP.mult, OP.add)
                for m in range(8):
                    s = qhat[:, m * TL:(m + 1) * TL]
                    nc.vector.tensor_mul(s, s, rstdb[:, 2 * N:2 * N + TL])
                    nc.vector.tensor_sub(s, s, rstdb[:, 2 * N + TL:2 * N + 2 * TL])
                    nc.vector.tensor_scalar(s, s, prm(0, m), prm(1, m),
                                            OP.mult, OP.add)

            # ---------- attention (heads in batches of 4) ----------
            with tc.tile_pool(name="pp", bufs=3) as ppool, \
                 tc.tile_pool(name="att", bufs=2) as attp, \
                 tc.tile_pool(name="wp2", bufs=8) as wpool2:
                for hb in range(4):
                    o_ps_list = []
                    for hh in range(4):
                        h = hb * 4 + hh
                        kth, hp = h // 2, (h % 2) * 64
                        q_ap = qhat[hp:hp + 64, kth * TL:(kth + 1) * TL]
                        o_ps = ps.tile([65, 512], F32, tag="ps")
                        o_ps_list.append(o_ps)
                        for tt in range(8):
                            s_ps = ps.tile([128, 512], F32, tag="ps")
                            nc.tensor.matmul(
                                s_ps[:],
                                khat[hp:hp + 64,
                                     kth * N + tt * 128: kth * N + (tt + 1) * 128],
                                q_ap, start=True, stop=True)
                            p_t = ppool.tile([128, 512], F32R, tag="p")
                            nc.scalar.activation(p_t[:], s_ps[:], AF.Exp,
                                                 scale=SCALE)
                            nc.tensor.matmul(
                                o_ps[:],
                                vful[:, tt * 1040 + h * 65: tt * 1040 + (h + 1) * 65],
                                p_t[:], start=(tt == 0), stop=(tt == 7))
                        nc.vector.tensor_copy(den4[32 * hh:32 * hh + 1, :],
                                              o_ps[64:65, :])
                    # batched reciprocal at rows 0,32,64,96: 1/d = exp(-ln d)
                    nc.scalar.activation(den4[:], den4[:], AF.Ln)
                    nc.scalar.activation(den4[:], den4[:], AF.Exp, scale=-1.0)
                    for hh in range(4):
                        h = hb * 4 + hh
                        kth, hp = h // 2, (h % 2) * 64
                        bc_ps = ps.tile([64, 512], F32, tag="ps")
                        nc.tensor.matmul(bc_ps[:],
                                         ones_big[32 * hh:32 * hh + 1, 0:64],
                                         den4[32 * hh:32 * hh + 1, :],
                                         start=True, stop=True,
                                         tile_position=(32 * hh, 0))
                        rb = attp.tile([64, 512], F32R, tag="rb")
                        nc.vector.tensor_copy(rb[:], bc_ps[:])
                        nc.vector.tensor_mul(
                            osb[hp:hp + 64, kth * TL:(kth + 1) * TL],
                            o_ps_list[hh][0:64, :], rb[:])

                # ---------- proj ----------
                for grp in range(2):
                    wp = [wpool2.tile([128, 512], F32R, tag="w2",
                                      name=f"wp{grp}_{i}") for i in range(KT)]
                    for kt in range(KT):
                        nc.sync.dma_start(
                            wp[kt][:],
                            projwT_d[kt * 128:(kt + 1) * 128,
                                     grp * 512:(grp + 1) * 512])
                    for mi in range(4):
                        m = grp * 4 + mi
                        acc = ps.tile([128, 512], F32, tag="ps")
                        for kt in range(KT):
                            nc.tensor.matmul(acc[:],
                                             wp[kt][:, mi * 128:(mi + 1) * 128],
                                             osb[:, kt * TL:(kt + 1) * TL],
                                             start=(kt == 0), stop=(kt == KT - 1))
                        ot = attp.tile([128, 512], F32, tag="ot")
                        nc.scalar.activation(ot[:], acc[:], AF.Identity,
                                             bias=prm(6, m))
                        nc.sync.dma_start(out_d[m * 128:(m + 1) * 128, :], ot[:])

    nc.compile()
    return nc


def _get_compiled():
    global _COMPILED
    if _COMPILED is None:
        _COMPILED = _build()
    return _COMPILED


def _host_prep(x, qkv_w, qkv_b, qn_w, qn_b, kn_w, kn_b, proj_w, proj_b):
    qkvwT = np.ascontiguousarray(np.asarray(qkv_w, np.float32).T)
    projwT = np.ascontiguousarray(np.asarray(proj_w, np.float32).T)
    qkv_b = np.asarray(qkv_b, np.float32)
    ws_q = np.asarray(qkv_w, np.float32)[0:C].sum(axis=0)
    ws_k = np.asarray(qkv_w, np.float32)[C:2 * C].sum(axis=0)
    wsums = np.zeros((128, 16), np.float32)
    for kt in range(8):
        wsums[:, kt * 2] = ws_q[kt * 128:(kt + 1) * 128]
        wsums[:, kt * 2 + 1] = ws_k[kt * 128:(kt + 1) * 128]
    bq = qkv_b[0:C].sum() / C
    bk = qkv_b[C:2 * C].sum() / C
    bsumC = np.array([[bq], [bk]], np.float32)
    params = np.zeros((128, 56), np.float32)
    for g, vec in enumerate([qn_w, qn_b, kn_w, kn_b,
                             qkv_b[0:C], qkv_b[C:2 * C], proj_b]):
        params[:, g * 8:(g + 1) * 8] = \
            np.asarray(vec, np.float32).reshape(8, 128).T
    bv = qkv_b[2 * C:3 * C].reshape(1, C).copy()
    selc = np.zeros((2, 256), np.float32)
    selc[0, 0:128] = 1.0
    selc[1, 128:256] = 1.0

    in_maps = []
    for c in range(N_CORES):
        b, half = c // 2, c % 2
        xb = np.asarray(x[b], np.float32)
        xr = np.roll(xb, -half * TL, axis=0)   # local tokens -> rows [0,512)
        xT = np.ascontiguousarray(xr.T)
        in_maps.append({
            "xT": xT, "qkvwT": qkvwT, "projwT": projwT, "wsums": wsums,
            "params": params, "bsumC": bsumC, "bv": bv, "selc": selc,
        })
    return in_maps


def _run(inputs, trace=False):
    from concourse.bass_utils import run_bass_kernel_spmd
    nc = _get_compiled()
    in_maps = _host_prep(**inputs)
    res = run_bass_kernel_spmd(nc, in_maps, core_ids=list(range(N_CORES)),
                               trace=trace)
    out = np.empty((B, N, C), np.float32)
    for c in range(N_CORES):
        b, half = c // 2, c % 2
        out[b, half * TL:(half + 1) * TL, :] = res.results[c]["out"].T
    return out, res


def kernel(**inputs):
    out, _ = _run(inputs, trace=False)
    return out


# revision 10
# speedup vs baseline: 1.1442x; 1.1442x over previous
"""Trainium2 Bass kernel for nn_Attention (dense transformer attention w/ QK-LayerNorm).

Sharding: sequence-parallel over 8 cores. Core c handles batch b = c//2,
token half h = c%2 (512 tokens). K/V are computed redundantly for the full
batch element on both cores of a pair (cheaper than the measured ~130us
pairwise AllGather); Q only for local tokens. No collectives.

Dataflow is transpose-free: Q/K are produced channel-major ([C, tok]) so the
QK-LayerNorm reductions over C become ones-vector matmuls on the partition
axis, and S^T = K_h Q_h^T tiles feed softmax-exp directly; V is produced
token-major with an appended ones-column per head so the PV matmul emits the
softmax denominator as an extra output row. All matmuls run in float32r
(TF32-rate on the PE at N>=256, fp32 storage).

Engine partition-access rule honored throughout: writes and ACT reads start
at 32-aligned partitions; only DVE reads use unaligned bases.
"""

import numpy as np

B, N, C = 4, 1024, 1024
H, D = 16, 64
LN_EPS = 1e-5
N_CORES = 8
TL = 512          # local tokens per core
KT = 8            # channel tiles of 128
SCALE = D ** -0.5

_COMPILED = None


def _build():
    import concourse.bacc as bacc
    import concourse.tile as tile
    import concourse.mybir as mybir

    F32 = mybir.dt.float32
    F32R = mybir.dt.float32r
    AF = mybir.ActivationFunctionType
    OP = mybir.AluOpType

    nc = bacc.Bacc("TRN2", target_bir_lowering=False, debug=False,
                   num_devices=N_CORES)

    # ---- I/O ----
    xT_d = nc.dram_tensor("xT", [C, N], F32R, kind="ExternalInput").ap()
    qkvwT_d = nc.dram_tensor("qkvwT", [C, 3 * C], F32R, kind="ExternalInput").ap()
    projwT_d = nc.dram_tensor("projwT", [C, C], F32R, kind="ExternalInput").ap()
    wsums_d = nc.dram_tensor("wsums", [128, 16], F32R, kind="ExternalInput").ap()
    params_d = nc.dram_tensor("params", [128, 56], F32, kind="ExternalInput").ap()
    bsum_d = nc.dram_tensor("bsumC", [2, 1], F32, kind="ExternalInput").ap()
    bv_d = nc.dram_tensor("bv", [1, C], F32R, kind="ExternalInput").ap()
    selc_d = nc.dram_tensor("selc", [2, 256], F32R, kind="ExternalInput").ap()
    out_d = nc.dram_tensor("out", [C, TL], F32, kind="ExternalOutput").ap()

    with tile.TileContext(nc) as tc:
        with tc.tile_pool(name="persist", bufs=1) as pers, \
             tc.tile_pool(name="sq", bufs=2) as sqp, \
             tc.tile_pool(name="small", bufs=1) as smallp, \
             tc.tile_pool(name="ps", bufs=8, space="PSUM") as ps:

            # ---------- persistent tiles ----------
            khat = pers.tile([128, KT * N], F32R, tag="khat")       # 4MB
            vful = pers.tile([128, 8 * 1040], F32R, tag="vful")     # 4.06MB
            qhat = pers.tile([128, KT * TL], F32R, tag="qhat")      # 2MB
            osb = pers.tile([128, KT * TL], F32R, tag="osb")        # 2MB
            rstdb = pers.tile([128, 2 * N + 2 * TL], F32R, tag="rstdb")  # 1.5MB

            params = smallp.tile([128, 56], F32, tag="params")
            wsums = smallp.tile([128, 16], F32R, tag="wsums")
            bsum = smallp.tile([2, 1], F32, tag="bsum")
            bv = smallp.tile([1, C], F32R, tag="bv")
            ones_c = smallp.tile([128, 1], F32R, tag="ones_c")
            ones_r = smallp.tile([1, 128], F32R, tag="ones_r")
            sel = smallp.tile([128, 4], F32R, tag="sel")  # e0=[1,0], e1=[0,1]
            # stats: column-packed [2, 512] slices, rows = (q, k)
            # slots: 0 sumsA 1 sumsB 2 ssqA 3 ssqB 4 mean 5 msq 6 rstd 7 mr
            st = smallp.tile([2, 7 * 512], F32, tag="st")
            stmm = smallp.tile([2, 2 * 512], F32R, tag="stmm")
            selc = smallp.tile([2, 256], F32R, tag="selc")
            ones_big = smallp.tile([128, 128], F32R, tag="ones_big")
            den4a = smallp.tile([128, 512], F32, tag="den4a")
            den4 = smallp.tile([128, 512], F32R, tag="den4")  # rows 0,32,64,96

            def sl(i):
                return st[:, i * 512:(i + 1) * 512]

            def smm(i):
                return stmm[:, i * 512:(i + 1) * 512]

            def prm(grp, kt):  # qn_w qn_b kn_w kn_b qb kb projb
                return params[:, grp * 8 + kt: grp * 8 + kt + 1]

            # ---------- load ----------
            nc.sync.dma_start(params[:], params_d[:])
            nc.sync.dma_start(wsums[:], wsums_d[:])
            nc.sync.dma_start(bsum[:], bsum_d[:])
            nc.sync.dma_start(bv[:], bv_d[:])
            nc.sync.dma_start(selc[:], selc_d[:])
            nc.vector.memset(ones_big[:].bitcast(F32), 1.0)
            nc.vector.memset(ones_c[:].bitcast(F32), 1.0)
            nc.vector.memset(ones_r[:].bitcast(F32), 1.0)
            nc.vector.memset(sel[:].bitcast(F32), 0.0)
            nc.vector.memset(sel[:, 0:1].bitcast(F32), 1.0)   # e0 col0
            nc.vector.memset(sel[:, 3:4].bitcast(F32), 1.0)   # e1 col1

            with tc.tile_pool(name="xp", bufs=1) as xpool, \
                 tc.tile_pool(name="wpool", bufs=8) as wpool:
                xT = xpool.tile([128, KT * N], F32R, tag="xT")      # 4MB
                xsrc = xT_d.rearrange("(a p) t -> p a t", p=128)
                nc.sync.dma_start(xT[:].rearrange("p (a t) -> p a t", a=KT), xsrc)

                # ---------- token sums of (q,k) ----------
                sums_ps = [ps.tile([2, 512], F32, tag="ps", name=f"sums_ps{i}")
                           for i in range(2)]
                for kt in range(KT):
                    for nh in range(2):
                        nc.tensor.matmul(
                            sums_ps[nh][:], wsums[:, kt * 2: kt * 2 + 2],
                            xT[:, kt * N + nh * 512: kt * N + (nh + 1) * 512],
                            start=(kt == 0), stop=(kt == KT - 1))
                nc.vector.tensor_copy(sl(0), sums_ps[0][:])
                nc.vector.tensor_copy(sl(1), sums_ps[1][:])

                # ssq accumulators (rows q,k): A=nh0(+q), B=nh1
                ssq_ps = [ps.tile([2, 512], F32, tag="ps", name=f"ssq_ps{i}")
                          for i in range(2)]

                # ---------- K phase ----------
                for grp in range(2):
                    wk = [wpool.tile([128, 512], F32R, tag="w",
                                     name=f"wk{grp}_{i}") for i in range(KT)]
                    for kt in range(KT):
                        nc.sync.dma_start(
                            wk[kt][:],
                            qkvwT_d[kt * 128:(kt + 1) * 128,
                                    C + grp * 512: C + (grp + 1) * 512])
                    for mi in range(4):
                        m = grp * 4 + mi
                        for nh in range(2):
                            acc = ps.tile([128, 512], F32, tag="ps")
                            for kt in range(KT):
                                nc.tensor.matmul(
                                    acc[:], wk[kt][:, mi * 128:(mi + 1) * 128],
                                    xT[:, kt * N + nh * 512: kt * N + (nh + 1) * 512],
                                    start=(kt == 0), stop=(kt == KT - 1))
                            dst = khat[:, m * N + nh * 512: m * N + (nh + 1) * 512]
                            nc.scalar.activation(dst, acc[:], AF.Identity,
                                                 bias=prm(5, m))
                            ksq = sqp.tile([128, 512], F32R, tag="sq")
                            nc.scalar.activation(ksq[:], dst.bitcast(F32),
                                                 AF.Square)
                            nc.tensor.matmul(ssq_ps[nh][:], sel[:, 2:4], ksq[:],
                                             start=(m == 0), stop=False,
                                             skip_group_check=True)

                # ---------- V phase ----------
                for nh in range(2):
                    wv = [wpool.tile([128, 512], F32R, tag="w",
                                     name=f"wv{nh}_{i}") for i in range(KT)]
                    for kt in range(KT):
                        nc.sync.dma_start(
                            wv[kt][:],
                            qkvwT_d[kt * 128:(kt + 1) * 128,
                                    2 * C + nh * 512: 2 * C + (nh + 1) * 512])
                    for mt in range(8):
                        base = mt * 1040
                        acc = ps.tile([128, 512], F32, tag="ps")
                        for kt in range(KT):
                            nc.tensor.matmul(
                                acc[:],
                                xT[:, kt * N + mt * 128: kt * N + (mt + 1) * 128],
                                wv[kt][:], start=(kt == 0), stop=False)
                        nc.tensor.matmul(acc[:], ones_r[:],
                                         bv[:, nh * 512:(nh + 1) * 512],
                                         start=False, stop=True)
                        dst = vful[:, base + nh * 8 * 65: base + (nh + 1) * 8 * 65]
                        nc.scalar.activation(
                            dst.rearrange("p (h e) -> p h e", h=8)[:, :, 0:64],
                            acc[:].rearrange("p (h e) -> p h e", h=8),
                            AF.Copy)
                for mt in range(8):
                    oc = vful[:, mt * 1040: (mt + 1) * 1040]
                    oc = oc.rearrange("p (h e) -> p h e", h=16)[:, :, 64:65]
                    nc.vector.memset(oc.bitcast(F32), 1.0)

                # ---------- Q phase ----------
                qsq = None
                for grp in range(2):
                    wq = [wpool.tile([128, 512], F32R, tag="w",
                                     name=f"wq{grp}_{i}") for i in range(KT)]
                    for kt in range(KT):
                        nc.sync.dma_start(
                            wq[kt][:],
                            qkvwT_d[kt * 128:(kt + 1) * 128,
                                    grp * 512:(grp + 1) * 512])
                    for mi in range(4):
                        m = grp * 4 + mi
                        acc = ps.tile([128, 512], F32, tag="ps")
                        for kt in range(KT):
                            nc.tensor.matmul(acc[:],
                                             wq[kt][:, mi * 128:(mi + 1) * 128],
                                             xT[:, kt * N: kt * N + TL],
                                             start=(kt == 0), stop=(kt == KT - 1))
                        dst = qhat[:, m * TL:(m + 1) * TL]
                        nc.scalar.activation(dst, acc[:], AF.Identity,
                                             bias=prm(4, m))
                        qsq = sqp.tile([128, 512], F32R, tag="sq")
                        nc.scalar.activation(qsq[:], dst.bitcast(F32), AF.Square)
                        nc.tensor.matmul(ssq_ps[0][:], sel[:, 0:2], qsq[:],
                                         start=False, stop=(m == 7),
                                         skip_group_check=True)
                # close ssq_ps[1] group (row 0 adds zeros)
                nc.tensor.matmul(ssq_ps[1][:], sel[:, 0:2], qsq[:],
                                 start=False, stop=True, skip_group_check=True)
                nc.vector.tensor_copy(sl(2), ssq_ps[0][:])
                nc.vector.tensor_copy(sl(3), ssq_ps[1][:])

                # ---------- stats chains (A: q + k-nh0, B: k-nh1) ----------
                def chain(sums_slot, ssq_slot):
                    nc.vector.tensor_scalar(sl(4), sums_slot, 1.0 / C,
                                            bsum[:], OP.mult, OP.add)
                    nc.vector.tensor_mul(sl(5), sl(4), sl(4))
                    nc.vector.tensor_scalar(sl(6), ssq_slot, 1.0 / C,
                                            LN_EPS, OP.mult, OP.add)
                    nc.vector.tensor_sub(sl(6), sl(6), sl(5))
                    nc.scalar.activation(sl(6), sl(6), AF.Ln)
                    nc.scalar.activation(smm(0), sl(6), AF.Exp, scale=-0.5)
                    nc.vector.tensor_mul(smm(1), sl(4),
                                         smm(0).bitcast(F32))

                def bcast(slot, row, dst_col):
                    bc_ps = ps.tile([128, 512], F32, tag="ps")
                    nc.tensor.matmul(bc_ps[:],
                                     selc[:, row * 128:(row + 1) * 128],
                                     smm(slot - 6), start=True, stop=True)
                    nc.vector.tensor_copy(
                        rstdb[:, dst_col:dst_col + 512].bitcast(F32), bc_ps[:])

                chain(sl(0), sl(2))                      # A: rows (q, k-nh0)
                bcast(6, 1, 0)                           # k rstd nh0
                bcast(7, 1, N)                           # k mr nh0
                bcast(6, 0, 2 * N)                       # q rstd
                bcast(7, 0, 2 * N + TL)                  # q mr
                chain(sl(1), sl(3))                      # B: rows (junk, k-nh1)
                bcast(6, 1, 512)                         # k rstd nh1
                bcast(7, 1, N + 512)                     # k mr nh1

                # ---------- normalize (in place) ----------
                with tc.tile_pool(name="ntmp", bufs=3) as ntp:
                    for m in range(8):
                        s = khat[:, m * N:(m + 1) * N]
                        t = ntp.tile([128, N], F32, tag="nt")
                        nc.vector.tensor_mul(t[:], s.bitcast(F32),
                                             rstdb[:, 0:N].bitcast(F32))
                        nc.vector.tensor_sub(t[:], t[:],
                                             rstdb[:, N:2 * N].bitcast(F32))
                        nc.scalar.activation(s, t[:], AF.Identity,
                                             scale=prm(2, m), bias=prm(3, m))
                    for m in range(8):
                        s = qhat[:, m * TL:(m + 1) * TL]
                        t = ntp.tile([128, TL], F32, tag="ntq")
                        nc.vector.tensor_mul(t[:], s.bitcast(F32),
                                             rstdb[:, 2 * N:2 * N + TL].bitcast(F32))
                        nc.vector.tensor_sub(
                            t[:], t[:],
                            rstdb[:, 2 * N + TL:2 * N + 2 * TL].bitcast(F32))
                        nc.scalar.activation(s, t[:], AF.Identity,
                                             scale=prm(0, m), bias=prm(1, m))

            # ---------- attention (head pairs, tt-interleaved) ----------
            with tc.tile_pool(name="pp", bufs=4) as ppool, \
                 tc.tile_pool(name="att", bufs=2) as attp, \
                 tc.tile_pool(name="wp2", bufs=16) as wpool2:
                # prefetch proj weights early (runs during attention)
                wp = [wpool2.tile([128, 512], F32R, tag="w2",
                                  name=f"wp_{i}") for i in range(2 * KT)]
                for grp in range(2):
                    for kt in range(KT):
                        nc.sync.dma_start(
                            wp[grp * KT + kt][:],
                            projwT_d[kt * 128:(kt + 1) * 128,
                                     grp * 512:(grp + 1) * 512])
                # zero-padded q staging: block0 rows 0-63 = even head,
                # block1 rows 64-127 = odd head; other halves stay zero
                qz = attp.tile([128, 1024], F32R, tag="qz", bufs=1)
                nc.vector.memset(qz[:].bitcast(F32), 0.0)

                for quad in range(4):
                    o_ps_all = []
                    for pr in range(2):
                        kth = quad * 2 + pr
                        hA, hB = 2 * kth, 2 * kth + 1
                        nc.vector.tensor_copy(
                            qz[0:64, 0:512],
                            qhat[0:64, kth * TL:(kth + 1) * TL])
                        nc.vector.tensor_copy(
                            qz[64:128, 512:1024],
                            qhat[64:128, kth * TL:(kth + 1) * TL])
                        o_psA = ps.tile([65, 512], F32, tag="ps", name=f"oA{kth}")
                        o_psB = ps.tile([65, 512], F32, tag="ps", name=f"oB{kth}")
                        o_ps_all += [o_psA, o_psB]
                        for tt in range(8):
                            ksl = khat[:, kth * N + tt * 128: kth * N + (tt + 1) * 128]
                            sA = ps.tile([128, 512], F32, tag="ps", name="sA")
                            nc.tensor.matmul(sA[:], ksl, qz[:, 0:512],
                                             start=True, stop=True)
                            sB = ps.tile([128, 512], F32, tag="ps", name="sB")
                            nc.tensor.matmul(sB[:], ksl, qz[:, 512:1024],
                                             start=True, stop=True)
                            pA = ppool.tile([128, 512], F32R, tag="p", name="pA")
                            nc.scalar.activation(pA[:], sA[:], AF.Exp, scale=SCALE)
                            pB = ppool.tile([128, 512], F32R, tag="p", name="pB")
                            nc.scalar.activation(pB[:], sB[:], AF.Exp, scale=SCALE)
                            nc.tensor.matmul(
                                o_psA[:],
                                vful[:, tt * 1040 + hA * 65: tt * 1040 + (hA + 1) * 65],
                                pA[:], start=(tt == 0), stop=(tt == 7))
                            nc.tensor.matmul(
                                o_psB[:],
                                vful[:, tt * 1040 + hB * 65: tt * 1040 + (hB + 1) * 65],
                                pB[:], start=(tt == 0), stop=(tt == 7))
                        nc.vector.tensor_copy(
                            den4a[64 * pr:64 * pr + 1, :], o_psA[64:65, :])
                        nc.vector.tensor_copy(
                            den4a[64 * pr + 32:64 * pr + 33, :], o_psB[64:65, :])
                    # batched reciprocal at rows 0,32,64,96: 1/d = exp(-ln d)
                    nc.scalar.activation(den4a[:], den4a[:], AF.Ln)
                    nc.scalar.activation(den4[:], den4a[:], AF.Exp, scale=-1.0)
                    for pr in range(2):
                        kth = quad * 2 + pr
                        for par in range(2):
                            hh = 2 * pr + par
                            hp = par * 64
                            bc_ps = ps.tile([64, 512], F32, tag="ps", name="bc")
                            nc.tensor.matmul(bc_ps[:],
                                             ones_big[32 * hh:32 * hh + 1, 0:64],
                                             den4[32 * hh:32 * hh + 1, :],
                                             start=True, stop=True,
                                             tile_position=(32 * hh, 0))
                            rb = attp.tile([64, 512], F32R, tag="rb")
                            nc.vector.tensor_copy(rb[:].bitcast(F32), bc_ps[:])
                            nc.vector.tensor_mul(
                                osb[hp:hp + 64, kth * TL:(kth + 1) * TL],
                                o_ps_all[2 * pr + par][0:64, :],
                                rb[:].bitcast(F32))

                # ---------- proj ----------
                for grp in range(2):
                    for mi in range(4):
                        m = grp * 4 + mi
                        acc = ps.tile([128, 512], F32, tag="ps")
                        for kt in range(KT):
                            nc.tensor.matmul(acc[:],
                                             wp[grp * KT + kt][:, mi * 128:(mi + 1) * 128],
                                             osb[:, kt * TL:(kt + 1) * TL],
                                             start=(kt == 0), stop=(kt == KT - 1))
                        ot = attp.tile([128, 512], F32, tag="ot")
                        nc.scalar.activation(ot[:], acc[:], AF.Identity,
                                             bias=prm(6, m))
                        nc.sync.dma_start(out_d[m * 128:(m + 1) * 128, :], ot[:])

    nc.compile()
    return nc


def _get_compiled():
    global _COMPILED
    if _COMPILED is None:
        _COMPILED = _build()
    return _COMPILED


def _host_prep(x, qkv_w, qkv_b, qn_w, qn_b, kn_w, kn_b, proj_w, proj_b):
    qkvwT = np.ascontiguousarray(np.asarray(qkv_w, np.float32).T)
    projwT = np.ascontiguousarray(np.asarray(proj_w, np.float32).T)
    qkv_b = np.asarray(qkv_b, np.float32)
    ws_q = np.asarray(qkv_w, np.float32)[0:C].sum(axis=0)
    ws_k = np.asarray(qkv_w, np.float32)[C:2 * C].sum(axis=0)
    wsums = np.zeros((128, 16), np.float32)
    for kt in range(8):
        wsums[:, kt * 2] = ws_q[kt * 128:(kt + 1) * 128]
        wsums[:, kt * 2 + 1] = ws_k[kt * 128:(kt + 1) * 128]
    bq = qkv_b[0:C].sum() / C
    bk = qkv_b[C:2 * C].sum() / C
    bsumC = np.array([[bq], [bk]], np.float32)
    params = np.zeros((128, 56), np.float32)
    for g, vec in enumerate([qn_w, qn_b, kn_w, kn_b,
                             qkv_b[0:C], qkv_b[C:2 * C], proj_b]):
        params[:, g * 8:(g + 1) * 8] = \
            np.asarray(vec, np.float32).reshape(8, 128).T
    bv = qkv_b[2 * C:3 * C].reshape(1, C).copy()
    selc = np.zeros((2, 256), np.float32)
    selc[0, 0:128] = 1.0
    selc[1, 128:256] = 1.0

    in_maps = []
    for c in range(N_CORES):
        b, half = c // 2, c % 2
        xb = np.asarray(x[b], np.float32)
        xr = np.roll(xb, -half * TL, axis=0)   # local tokens -> rows [0,512)
        xT = np.ascontiguousarray(xr.T)
        in_maps.append({
            "xT": xT, "qkvwT": qkvwT, "projwT": projwT, "wsums": wsums,
            "params": params, "bsumC": bsumC, "bv": bv, "selc": selc,
        })
    return in_maps


def _run(inputs, trace=False):
    from concourse.bass_utils import run_bass_kernel_spmd
    nc = _get_compiled()
    in_maps = _host_prep(**inputs)
    res = run_bass_kernel_spmd(nc, in_maps, core_ids=list(range(N_CORES)),
                               trace=trace)
    out = np.empty((B, N, C), np.float32)
    for c in range(N_CORES):
        b, half = c // 2, c % 2
        out[b, half * TL:(half + 1) * TL, :] = res.results[c]["out"].T
    return out, res


def kernel(**inputs):
    out, _ = _run(inputs, trace=False)
    return out


# revision 14
# speedup vs baseline: 1.1673x; 1.0202x over previous
"""Trainium2 Bass kernel for nn_Attention (dense transformer attention w/ QK-LayerNorm).

Sharding: sequence-parallel over 8 cores. Core c handles batch b = c//2,
token half h = c%2 (512 tokens). K/V are computed redundantly for the full
batch element on both cores of a pair (cheaper than the measured ~130us
pairwise AllGather); Q only for local tokens. No collectives.

Dataflow is transpose-free: Q/K are produced channel-major ([C, tok]) so the
QK-LayerNorm reductions over C become ones-vector matmuls on the partition
axis, and S^T = K_h Q_h^T tiles feed softmax-exp directly; V is produced
token-major with an appended ones-column per head so the PV matmul emits the
softmax denominator as an extra output row. All matmuls run in float32r
(TF32-rate on the PE at N>=256, fp32 storage).

Engine partition-access rule honored throughout: writes and ACT reads start
at 32-aligned partitions; only DVE reads use unaligned bases.
"""

import numpy as np

B, N, C = 4, 1024, 1024
H, D = 16, 64
LN_EPS = 1e-5
N_CORES = 8
TL = 512          # local tokens per core
KT = 8            # channel tiles of 128
SCALE = D ** -0.5

_COMPILED = None


def _build():
    import concourse.bacc as bacc
    import concourse.tile as tile
    import concourse.mybir as mybir

    F32 = mybir.dt.float32
    F32R = mybir.dt.float32r
    AF = mybir.ActivationFunctionType
    OP = mybir.AluOpType

    nc = bacc.Bacc("TRN2", target_bir_lowering=False, debug=False,
                   num_devices=N_CORES)

    # ---- I/O ----
    xT_d = nc.dram_tensor("xT", [C, N], F32R, kind="ExternalInput").ap()
    qkvwT_d = nc.dram_tensor("qkvwT", [C, 3 * C], F32R, kind="ExternalInput").ap()
    projwT_d = nc.dram_tensor("projwT", [C, C], F32R, kind="ExternalInput").ap()
    wsums_d = nc.dram_tensor("wsums", [128, 16], F32R, kind="ExternalInput").ap()
    params_d = nc.dram_tensor("params", [128, 56], F32, kind="ExternalInput").ap()
    bsum_d = nc.dram_tensor("bsumC", [2, 1], F32, kind="ExternalInput").ap()
    selc_d = nc.dram_tensor("selc", [2, 256], F32R, kind="ExternalInput").ap()
    out_d = nc.dram_tensor("out", [C, TL], F32, kind="ExternalOutput").ap()

    with tile.TileContext(nc) as tc:
        with tc.tile_pool(name="persist", bufs=1) as pers, \
             tc.tile_pool(name="sq", bufs=2) as sqp, \
             tc.tile_pool(name="small", bufs=1) as smallp, \
             tc.tile_pool(name="ps", bufs=8, space="PSUM") as ps:

            # ---------- persistent tiles ----------
            khat = pers.tile([128, KT * N], F32R, tag="khat")       # 4MB
            vful = pers.tile([128, 8 * 1040], F32R, tag="vful")     # 4.06MB
            qhat = pers.tile([128, KT * TL], F32R, tag="qhat")      # 2MB

            params = smallp.tile([128, 56], F32, tag="params")
            wsums = smallp.tile([128, 16], F32R, tag="wsums")
            bsum = smallp.tile([2, 1], F32, tag="bsum")
            sel = smallp.tile([128, 4], F32R, tag="sel")  # e0=[1,0], e1=[0,1]
            # stats: column-packed [2, 512] slices, rows = (q, k)
            # slots: 0 sumsA 1 sumsB 2 ssqA 3 ssqB 4 mean 5 msq 6 rstd 7 mr
            st = smallp.tile([2, 7 * 512], F32, tag="st")
            stmm = smallp.tile([2, 2 * 512], F32R, tag="stmm")
            selc = smallp.tile([2, 256], F32R, tag="selc")
            ones_big = smallp.tile([128, 128], F32R, tag="ones_big")
            den4a = smallp.tile([128, 512], F32, tag="den4a")
            den4 = smallp.tile([128, 512], F32R, tag="den4")  # rows 0,32,64,96

            def sl(i):
                return st[:, i * 512:(i + 1) * 512]

            def smm(i):
                return stmm[:, i * 512:(i + 1) * 512]

            def prm(grp, kt):  # qn_w qn_b kn_w kn_b qb kb projb
                return params[:, grp * 8 + kt: grp * 8 + kt + 1]

            # ---------- load ----------
            nc.sync.dma_start(params[:], params_d[:])
            nc.sync.dma_start(wsums[:], wsums_d[:])
            nc.sync.dma_start(bsum[:], bsum_d[:])
            nc.sync.dma_start(selc[:], selc_d[:])
            nc.vector.memset(ones_big[:].bitcast(F32), 1.0)
            nc.vector.memset(sel[:].bitcast(F32), 0.0)
            nc.vector.memset(sel[:, 0:1].bitcast(F32), 1.0)   # e0 col0
            nc.vector.memset(sel[:, 3:4].bitcast(F32), 1.0)   # e1 col1

            with tc.tile_pool(name="xp", bufs=1) as xpool, \
                 tc.tile_pool(name="wpool", bufs=16) as wpool:
                xT = xpool.tile([128, KT * N], F32R, tag="xT")      # 4MB
                rstdb = xpool.tile([128, 2 * N + 2 * TL], F32R, tag="rstdb")
                xsrc = xT_d.rearrange("(a p) t -> p a t", p=128)
                nc.sync.dma_start(xT[:].rearrange("p (a t) -> p a t", a=KT), xsrc)

                # ---------- token sums of (q,k) ----------
                sums_ps = [ps.tile([2, 512], F32, tag="ps", name=f"sums_ps{i}")
                           for i in range(2)]
                for kt in range(KT):
                    for nh in range(2):
                        nc.tensor.matmul(
                            sums_ps[nh][:], wsums[:, kt * 2: kt * 2 + 2],
                            xT[:, kt * N + nh * 512: kt * N + (nh + 1) * 512],
                            start=(kt == 0), stop=(kt == KT - 1))
                nc.vector.tensor_copy(sl(0), sums_ps[0][:])
                nc.vector.tensor_copy(sl(1), sums_ps[1][:])

                # ssq accumulators (rows q,k): A=nh0, B=nh1 (k only)
                ssq_ps = [ps.tile([2, 512], F32, tag="ps", name=f"ssq_ps{i}")
                          for i in range(2)]

                # ---------- K phase ----------
                for grp in range(2):
                    wk = [wpool.tile([128, 512], F32R, tag="w",
                                     name=f"wk{grp}_{i}") for i in range(KT)]
                    for kt in range(KT):
                        nc.sync.dma_start(
                            wk[kt][:],
                            qkvwT_d[kt * 128:(kt + 1) * 128,
                                    C + grp * 512: C + (grp + 1) * 512])
                    for mi in range(4):
                        m = grp * 4 + mi
                        for nh in range(2):
                            acc = ps.tile([128, 512], F32, tag="ps")
                            for kt in range(KT):
                                nc.tensor.matmul(
                                    acc[:], wk[kt][:, mi * 128:(mi + 1) * 128],
                                    xT[:, kt * N + nh * 512: kt * N + (nh + 1) * 512],
                                    start=(kt == 0), stop=(kt == KT - 1))
                            dst = khat[:, m * N + nh * 512: m * N + (nh + 1) * 512]
                            nc.scalar.activation(dst, acc[:], AF.Identity,
                                                 bias=prm(5, m))
                            ksq = sqp.tile([128, 512], F32R, tag="sq")
                            nc.scalar.activation(ksq[:], dst.bitcast(F32),
                                                 AF.Square)
                            nc.tensor.matmul(ssq_ps[nh][:], sel[:, 2:4], ksq[:],
                                             start=(m == 0), stop=(m == 7),
                                             skip_group_check=True)

                def chain(sums_slot, ssq_slot):
                    nc.vector.tensor_scalar(sl(4), sums_slot, 1.0 / C,
                                            bsum[:], OP.mult, OP.add)
                    nc.vector.tensor_mul(sl(5), sl(4), sl(4))
                    nc.vector.tensor_scalar(sl(6), ssq_slot, 1.0 / C,
                                            LN_EPS, OP.mult, OP.add)
                    nc.vector.tensor_sub(sl(6), sl(6), sl(5))
                    nc.vector.tensor_scalar_max(sl(6), sl(6), 1e-20)
                    nc.scalar.activation(sl(6), sl(6), AF.Ln)
                    nc.scalar.activation(smm(0), sl(6), AF.Exp, scale=-0.5)
                    nc.vector.tensor_mul(smm(1), sl(4),
                                         smm(0).bitcast(F32))

                def bcast(slot, row, dst_col):
                    bc_ps = ps.tile([128, 512], F32, tag="ps")
                    nc.tensor.matmul(bc_ps[:],
                                     selc[:, row * 128:(row + 1) * 128],
                                     smm(slot - 6), start=True, stop=True)
                    nc.vector.tensor_copy(
                        rstdb[:, dst_col:dst_col + 512].bitcast(F32), bc_ps[:])

                # k stats + normalize now (overlaps V/Q matmuls on PE)
                nc.vector.tensor_copy(sl(2), ssq_ps[0][:])
                nc.vector.tensor_copy(sl(3), ssq_ps[1][:])
                chain(sl(0), sl(2))
                bcast(6, 1, 0)
                bcast(7, 1, N)
                chain(sl(1), sl(3))
                bcast(6, 1, 512)
                bcast(7, 1, N + 512)
                with tc.tile_pool(name="ntmp", bufs=3) as ntp:
                    for m in range(8):
                        s = khat[:, m * N:(m + 1) * N]
                        t = ntp.tile([128, N], F32, tag="nt")
                        nc.vector.tensor_mul(t[:], s.bitcast(F32),
                                             rstdb[:, 0:N].bitcast(F32))
                        nc.vector.tensor_sub(t[:], t[:],
                                             rstdb[:, N:2 * N].bitcast(F32))
                        nc.scalar.activation(s, t[:], AF.Identity,
                                             scale=prm(2, m), bias=prm(3, m))

                # ---------- V phase ----------
                for nh in range(2):
                    wv = [wpool.tile([128, 512], F32R, tag="w",
                                     name=f"wv{nh}_{i}") for i in range(KT)]
                    for kt in range(KT):
                        nc.sync.dma_start(
                            wv[kt][:],
                            qkvwT_d[kt * 128:(kt + 1) * 128,
                                    2 * C + nh * 512: 2 * C + (nh + 1) * 512])
                    for mt in range(8):
                        base = mt * 1040
                        acc = ps.tile([128, 512], F32, tag="ps")
                        for kt in range(KT):
                            nc.tensor.matmul(
                                acc[:],
                                xT[:, kt * N + mt * 128: kt * N + (mt + 1) * 128],
                                wv[kt][:], start=(kt == 0), stop=(kt == KT - 1))
                        dst = vful[:, base + nh * 8 * 65: base + (nh + 1) * 8 * 65]
                        nc.scalar.activation(
                            dst.rearrange("p (h e) -> p h e", h=8)[:, :, 0:64],
                            acc[:].rearrange("p (h e) -> p h e", h=8),
                            AF.Copy)
                for mt in range(8):
                    oc = vful[:, mt * 1040: (mt + 1) * 1040]
                    oc = oc.rearrange("p (h e) -> p h e", h=16)[:, :, 64:65]
                    nc.vector.memset(oc.bitcast(F32), 1.0)

                # ---------- Q phase ----------
                ssqQ_ps = ps.tile([2, 512], F32, tag="ps", name="ssqQ_ps")
                for grp in range(2):
                    wq = [wpool.tile([128, 512], F32R, tag="w",
                                     name=f"wq{grp}_{i}") for i in range(KT)]
                    for kt in range(KT):
                        nc.sync.dma_start(
                            wq[kt][:],
                            qkvwT_d[kt * 128:(kt + 1) * 128,
                                    grp * 512:(grp + 1) * 512])
                    for mi in range(4):
                        m = grp * 4 + mi
                        acc = ps.tile([128, 512], F32, tag="ps")
                        for kt in range(KT):
                            nc.tensor.matmul(acc[:],
                                             wq[kt][:, mi * 128:(mi + 1) * 128],
                                             xT[:, kt * N: kt * N + TL],
                                             start=(kt == 0), stop=(kt == KT - 1))
                        dst = qhat[:, m * TL:(m + 1) * TL]
                        nc.scalar.activation(dst, acc[:], AF.Identity,
                                             bias=prm(4, m))
                        qsq = sqp.tile([128, 512], F32R, tag="sq")
                        nc.scalar.activation(qsq[:], dst.bitcast(F32), AF.Square)
                        nc.tensor.matmul(ssqQ_ps[:], sel[:, 0:2], qsq[:],
                                         start=(m == 0), stop=(m == 7),
                                         skip_group_check=True)

                # q stats + normalize
                nc.vector.tensor_copy(sl(2), ssqQ_ps[:])
                chain(sl(0), sl(2))
                bcast(6, 0, 2 * N)
                bcast(7, 0, 2 * N + TL)
                with tc.tile_pool(name="ntq", bufs=3) as ntq:
                    for m in range(8):
                        s = qhat[:, m * TL:(m + 1) * TL]
                        t = ntq.tile([128, TL], F32, tag="ntq")
                        nc.vector.tensor_mul(t[:], s.bitcast(F32),
                                             rstdb[:, 2 * N:2 * N + TL].bitcast(F32))
                        nc.vector.tensor_sub(
                            t[:], t[:],
                            rstdb[:, 2 * N + TL:2 * N + 2 * TL].bitcast(F32))
                        nc.scalar.activation(s, t[:], AF.Identity,
                                             scale=prm(0, m), bias=prm(1, m))

            # ---------- attention (head pairs, tt-interleaved) ----------
            with tc.tile_pool(name="pp", bufs=4) as ppool, \
                 tc.tile_pool(name="att", bufs=2) as attp, \
                 tc.tile_pool(name="osbp", bufs=1) as osbp, \
                 tc.tile_pool(name="wp2", bufs=16) as wpool2:
                osb = osbp.tile([128, KT * TL], F32R, tag="osb")    # 2MB
                # prefetch proj weights early (runs during attention)
                wp = [wpool2.tile([128, 512], F32R, tag="w2",
                                  name=f"wp_{i}") for i in range(2 * KT)]
                for grp in range(2):
                    for kt in range(KT):
                        nc.sync.dma_start(
                            wp[grp * KT + kt][:],
                            projwT_d[kt * 128:(kt + 1) * 128,
                                     grp * 512:(grp + 1) * 512])
                # zero-padded q staging: block0 rows 0-63 = even head,
                # block1 rows 64-127 = odd head; other halves stay zero
                qz = attp.tile([128, 1024], F32R, tag="qz", bufs=1)
                nc.vector.memset(qz[:].bitcast(F32), 0.0)

                for kth in range(8):
                    hA, hB = 2 * kth, 2 * kth + 1
                    nc.vector.tensor_copy(
                        qz[0:64, 0:512],
                        qhat[0:64, kth * TL:(kth + 1) * TL])
                    nc.vector.tensor_copy(
                        qz[64:128, 512:1024],
                        qhat[64:128, kth * TL:(kth + 1) * TL])
                    o_psA = ps.tile([65, 512], F32, tag="ps", name=f"oA{kth}")
                    o_psB = ps.tile([65, 512], F32, tag="ps", name=f"oB{kth}")
                    for tt in range(8):
                        ksl = khat[:, kth * N + tt * 128: kth * N + (tt + 1) * 128]
                        sA = ps.tile([128, 512], F32, tag="ps", name="sA")
                        nc.tensor.matmul(sA[:], ksl, qz[:, 0:512],
                                         start=True, stop=True)
                        sB = ps.tile([128, 512], F32, tag="ps", name="sB")
                        nc.tensor.matmul(sB[:], ksl, qz[:, 512:1024],
                                         start=True, stop=True)
                        pA = ppool.tile([128, 512], F32R, tag="p", name="pA")
                        nc.scalar.activation(pA[:], sA[:], AF.Exp, scale=SCALE)
                        pB = ppool.tile([128, 512], F32R, tag="p", name="pB")
                        nc.scalar.activation(pB[:], sB[:], AF.Exp, scale=SCALE)
                        nc.tensor.matmul(
                            o_psA[:],
                            vful[:, tt * 1040 + hA * 65: tt * 1040 + (hA + 1) * 65],
                            pA[:], start=(tt == 0), stop=(tt == 7))
                        nc.tensor.matmul(
                            o_psB[:],
                            vful[:, tt * 1040 + hB * 65: tt * 1040 + (hB + 1) * 65],
                            pB[:], start=(tt == 0), stop=(tt == 7))
                    # per-pair denominator reciprocal: 1/d = exp(-ln d)
                    dna = attp.tile([64, 512], F32, tag="dna")
                    dnb = attp.tile([64, 512], F32R, tag="dnb")
                    nc.vector.tensor_copy(dna[0:1, :], o_psA[64:65, :])
                    nc.vector.tensor_copy(dna[32:33, :], o_psB[64:65, :])
                    nc.scalar.activation(dna[:], dna[:], AF.Ln)
                    nc.scalar.activation(dnb[:], dna[:], AF.Exp, scale=-1.0)
                    for par, o_ps in ((0, o_psA), (1, o_psB)):
                        hp = par * 64
                        bc_ps = ps.tile([64, 512], F32, tag="ps", name="bc")
                        nc.tensor.matmul(bc_ps[:],
                                         ones_big[32 * par:32 * par + 1, 0:64],
                                         dnb[32 * par:32 * par + 1, :],
                                         start=True, stop=True,
                                         tile_position=(32 * par, 0))
                        rb = attp.tile([64, 512], F32R, tag="rb")
                        nc.vector.tensor_copy(rb[:].bitcast(F32), bc_ps[:])
                        nc.vector.tensor_mul(
                            osb[hp:hp + 64, kth * TL:(kth + 1) * TL],
                            o_ps[0:64, :], rb[:].bitcast(F32))

                # ---------- proj ----------
                for grp in range(2):
                    for mi in range(4):
                        m = grp * 4 + mi
                        acc = ps.tile([128, 512], F32, tag="ps")
                        for kt in range(KT):
                            nc.tensor.matmul(acc[:],
                                             wp[grp * KT + kt][:, mi * 128:(mi + 1) * 128],
                                             osb[:, kt * TL:(kt + 1) * TL],
                                             start=(kt == 0), stop=(kt == KT - 1))
                        ot = attp.tile([128, 512], F32, tag="ot")
                        nc.scalar.activation(ot[:], acc[:], AF.Identity,
                                             bias=prm(6, m))
                        nc.sync.dma_start(out_d[m * 128:(m + 1) * 128, :], ot[:])

    nc.compile()
    return nc


def _get_compiled():
    global _COMPILED
    if _COMPILED is None:
        _COMPILED = _build()
    return _COMPILED


def _host_prep(x, qkv_w, qkv_b, qn_w, qn_b, kn_w, kn_b, proj_w, proj_b):
    qkvwT = np.ascontiguousarray(np.asarray(qkv_w, np.float32).T)
    projwT = np.ascontiguousarray(np.asarray(proj_w, np.float32).T)
    qkv_b = np.asarray(qkv_b, np.float32)
    ws_q = np.asarray(qkv_w, np.float32)[0:C].sum(axis=0)
    ws_k = np.asarray(qkv_w, np.float32)[C:2 * C].sum(axis=0)
    wsums = np.zeros((128, 16), np.float32)
    for kt in range(8):
        wsums[:, kt * 2] = ws_q[kt * 128:(kt + 1) * 128]
        wsums[:, kt * 2 + 1] = ws_k[kt * 128:(kt + 1) * 128]
    bq = qkv_b[0:C].sum() / C
    bk = qkv_b[C:2 * C].sum() / C
    bsumC = np.array([[bq], [bk]], np.float32)
    params = np.zeros((128, 56), np.float32)
    proj_b2 = np.asarray(proj_b, np.float32) + \
        np.asarray(proj_w, np.float32) @ qkv_b[2 * C:3 * C]
    for g, vec in enumerate([qn_w, qn_b, kn_w, kn_b,
                             qkv_b[0:C], qkv_b[C:2 * C], proj_b2]):
        params[:, g * 8:(g + 1) * 8] = \
            np.asarray(vec, np.float32).reshape(8, 128).T
    selc = np.zeros((2, 256), np.float32)
    selc[0, 0:128] = 1.0
    selc[1, 128:256] = 1.0

    in_maps = []
    for c in range(N_CORES):
        b, half = c // 2, c % 2
        xb = np.asarray(x[b], np.float32)
        xr = np.roll(xb, -half * TL, axis=0)   # local tokens -> rows [0,512)
        xT = np.ascontiguousarray(xr.T)
        in_maps.append({
            "xT": xT, "qkvwT": qkvwT, "projwT": projwT, "wsums": wsums,
            "params": params, "bsumC": bsumC, "selc": selc,
        })
    return in_maps


def _run(inputs, trace=False):
    from concourse.bass_utils import run_bass_kernel_spmd
    nc = _get_compiled()
    in_maps = _host_prep(**inputs)
    res = run_bass_kernel_spmd(nc, in_maps, core_ids=list(range(N_CORES)),
                               trace=trace)
    out = np.empty((B, N, C), np.float32)
    for c in range(N_CORES):
        b, half = c // 2, c % 2
        out[b, half * TL:(half + 1) * TL, :] = res.results[c]["out"].T
    return out, res


def kernel(**inputs):
    out, _ = _run(inputs, trace=False)
    return out
